# revision 42
# baseline (speedup 1.0000x reference)
"""BEV deformable-attention encoder layer on 8 Trainium2 NeuronCores.

Sharding: one offset-group/head per core (tensor-parallel over the (b*g)=8
leading dim, per the sharding hint). Host does the tiny irregular prep
(q/k/v 1x1 projections, offset conv network, bilinear grid-sample, signed-log
coordinate tables) with vectorized numpy/BLAS; each core runs the dominant
compute: the CPB pairwise MLP (2->64->64->1 over 1600*100 pairs/core),
attention logits, softmax and attn@V. Cores return outT (64,1600) in bf16;
host applies the final 1x1 output projection (one 256x512x1600 sgemm) — the
tensor-parallel un-shard — and adds b_out.

Device-side CPB structure (all exact math, no per-pair upload):
 - layer 1: z1 = W0x*u + W0y*v + b0 over pairs is an outer sum; computed as
   two PSUM-accumulated selector matmuls per (j, window):
     z1 = U_j^T-free @ S_tile + V_j @ S_rep
   where U_j = u[j,:] (x) w0x, V_j = v[j,:] (x) w0y + 1 (x) b0 are built on
   device by tiny K=1/K=2 outer-product matmuls, and S_tile/S_rep are static
   0/1 selectors built on device (identity tiling + row memsets).
 - layer 2: |w2| is folded into W1/b1 rows, so layer 3 reduces with a +-1
   sign vector: bias = sum_c sign(w2_c) * relu(z2''_c).
 - layer 3: the sign vector is placed in column j of an otherwise-zero
   (64,100) weight (a slice of a (64,199) "sliding" matrix), so each j's
   matmul accumulates bias directly into the (100, win) attention-logit PSUM
   tile, pre-added to q@k^T. No elementwise scatter, no max-subtraction
   needed (|logits| < 2).

Invocation layer: a cached jax.jit(shard_map(bass_exec)) callable (built
once per process) with resident zero buffers — avoids per-call re-tracing
and the donated zero-output upload. A BIR legalization pass splits >1
sync-wait instructions (this walrus build allows one wait slot per
instruction).
"""

import json
import math
import numpy as np
import ml_dtypes

_BF16 = np.dtype(ml_dtypes.bfloat16)

D_MODEL, HEADS, GROUPS, DIM_HEAD = 256, 8, 8, 64
INNER = HEADS * DIM_HEAD
OFF_DIMS = INNER // GROUPS            # 64
DF, OFF_SCALE, KS, PAD = 4, 4.0, 6, 1
NUM_LAYERS = 6
SCALE = DIM_HEAD ** -0.5
B, H, W = 1, 40, 40
HP = WP = 10
I, J = H * W, HP * WP                 # 1600 queries, 100 keys per core
N_CORES = 8
WIN = 400                             # i-window processed per PSUM tile
NWIN = I // WIN                       # 4


# ---------------------------------------------------------------------------
# BIR legalization: this walrus build rejects instructions with >1 sync wait.
# Split extra waits into preceding single-wait EventSemaphore instructions on
# the same engine (semantically identical: the engine blocks in order).
# ---------------------------------------------------------------------------

def _legalize_sync_waits(bir_json_bytes):
    bir = json.loads(bir_json_bytes)
    changed = False
    for fn in bir.get('functions', []):
        for blk in fn.get('blocks', []):
            out = []
            for ins in blk.get('instructions', []):
                si = ins.get('sync_info') or {}
                ow = si.get('on_wait') or []
                if len(ow) > 1:
                    changed = True
                    for k, w in enumerate(ow[:-1]):
                        out.append({
                            'debug': ins.get('debug', 0),
                            'engine': ins['engine'],
                            'ins': [],
                            'name': f"{ins['name']}_w{k}",
                            'opcode': 'EventSemaphore',
                            'outs': [],
                            'sync_info': {'on_update': [], 'on_wait': [w]},
                        })
                    si['on_wait'] = ow[-1:]
                    ins['sync_info'] = si
                out.append(ins)
            blk['instructions'] = out
    return json.dumps(bir).encode() if changed else bir_json_bytes


def _install_birfix():
    import os
    import shutil
    import hashlib
    import concourse.bass2jax as b2j
    import concourse.bass_utils as bu
    if getattr(b2j, '_birfix_installed', False):
        return
    orig = bu.compile_bir_kernel
    cache_dir = os.path.expanduser('~/.cache/bassneff')

    def wrapped(ant_bir_str, compile_dir_path, neff_name='kernel.neff', **kw):
        if isinstance(ant_bir_str, str):
            ant_bir_str = ant_bir_str.encode()
        bir = _legalize_sync_waits(ant_bir_str)
        key = hashlib.sha256(bir + neff_name.encode()).hexdigest()[:32]
        cached = os.path.join(cache_dir, f'{key}.neff')
        dst = os.path.join(compile_dir_path, neff_name)
        if os.path.exists(cached):
            shutil.copyfile(cached, dst)
            return dst
        neff_file = orig(bir, compile_dir_path, neff_name=neff_name, **kw)
        try:
            os.makedirs(cache_dir, exist_ok=True)
            shutil.copyfile(neff_file, cached + '.tmp')
            os.replace(cached + '.tmp', cached)
        except OSError:
            pass
        return neff_file

    b2j.compile_bir_kernel = wrapped
    b2j._birfix_installed = True


# ---------------------------------------------------------------------------
# Host prep (vectorized numpy)
# ---------------------------------------------------------------------------

_BUF = {
    'qp': np.zeros((GROUPS, OFF_DIMS, H + 2 * PAD, W + 2 * PAD), np.float32),
    'conv': np.empty((GROUPS, OFF_DIMS, HP, WP), np.float32),
    'ctmp': np.empty((GROUPS, OFF_DIMS, HP, WP), np.float32),
}


def _gelu_exact(x):
    try:
        from scipy.special import erf
        return 0.5 * x * (1.0 + erf(x / math.sqrt(2.0)))
    except ImportError:
        ef = np.vectorize(math.erf, otypes=[np.float64])
        return 0.5 * x * (1.0 + ef(x / math.sqrt(2.0))).astype(np.float32)


def _host_prep(bev_feat, wq, wk, wv, w_off1, b_off1, w_off2,
               cpb_w0, cpb_b0, cpb_w1, cpb_b1, cpb_w2, cpb_b2, w_out, b_out):
    l = NUM_LAYERS - 1
    f32 = np.float32
    x = np.asarray(bev_feat, f32)[0].reshape(GROUPS, D_MODEL // GROUPS, I)  # (8,32,1600)
    wq5 = np.asarray(wq[l], f32).reshape(GROUPS, OFF_DIMS, D_MODEL // GROUPS)
    wk5 = np.asarray(wk[l], f32).reshape(GROUPS, OFF_DIMS, D_MODEL // GROUPS)
    wv5 = np.asarray(wv[l], f32).reshape(GROUPS, OFF_DIMS, D_MODEL // GROUPS)

    q8 = np.matmul(wq5, x)                                   # (8,64,1600)

    # offset network: depthwise 6x6 stride-4 conv, GELU, 1x1, tanh*4
    w1 = np.asarray(w_off1[l], f32)                          # (64,1,6,6)
    b1o = np.asarray(b_off1[l], f32)
    qp = _BUF['qp']                # borders stay zero; interior overwritten
    qp[:, :, PAD:PAD + H, PAD:PAD + W] = q8.reshape(GROUPS, OFF_DIMS, H, W)
    conv = _BUF['conv']
    tmp = _BUF['ctmp']
    first = True
    for ky in range(KS):
        for kx in range(KS):
            dst = conv if first else tmp
            np.multiply(qp[:, :, ky:ky + DF * HP:DF, kx:kx + DF * WP:DF],
                        w1[None, :, 0, ky, kx, None, None], out=dst)
            if not first:
                conv += tmp
            first = False
    conv += b1o[None, :, None, None]
    hgel = _gelu_exact(conv).astype(f32).reshape(GROUPS, OFF_DIMS, J)
    w_off2l = np.asarray(w_off2[l], f32)                     # (2,64)
    off = np.tanh(np.einsum('oc,gcj->goj', w_off2l, hgel)) * OFF_SCALE  # (8,2,100)

    ysp, xsp = np.meshgrid(np.arange(HP, dtype=f32), np.arange(WP, dtype=f32),
                           indexing='ij')
    base = np.stack([xsp, ysp]).reshape(2, J)                # (2,100)
    vg = base[None] + off                                    # (8,2,100)
    gkvx = (2.0 * vg[:, 0] / (HP - 1) - 1.0).astype(f32)     # (8,100)
    gkvy = (2.0 * vg[:, 1] / (WP - 1) - 1.0).astype(f32)

    # bilinear grid-sample of x at gkv (zeros padding, align_corners=False)
    gx = ((gkvx + 1.0) * W - 1.0) * 0.5
    gy = ((gkvy + 1.0) * H - 1.0) * 0.5
    x0 = np.floor(gx); y0 = np.floor(gy)
    wx1 = gx - x0; wy1 = gy - y0
    kv = np.zeros((GROUPS, D_MODEL // GROUPS, J), f32)
    for dx, dy, wgt in ((0, 0, (1 - wx1) * (1 - wy1)), (1, 0, wx1 * (1 - wy1)),
                        (0, 1, (1 - wx1) * wy1), (1, 1, wx1 * wy1)):
        xi = x0 + dx; yi = y0 + dy
        valid = (xi >= 0) & (xi <= W - 1) & (yi >= 0) & (yi <= H - 1)
        xc = np.clip(xi, 0, W - 1).astype(np.int64)
        yc = np.clip(yi, 0, H - 1).astype(np.int64)
        idx = (yc * W + xc)[:, None, :]                      # (8,1,100)
        kv += np.take_along_axis(x, idx, axis=2) * (wgt * valid)[:, None, :]

    k8 = np.matmul(wk5, kv)                                  # (8,64,100)
    v8 = np.matmul(wv5, kv)

    # signed-log coordinate tables: u[g,j,ix], v[g,j,iy]
    gqx = (2.0 * np.arange(W, dtype=f32) / (W - 1) - 1.0)    # ix axis (x coord)
    gqy = (2.0 * np.arange(H, dtype=f32) / (H - 1) - 1.0)
    du = gqx[None, None, :] - gkvx[:, :, None]               # (8,100,40)
    dv = gqy[None, None, :] - gkvy[:, :, None]
    uu = np.sign(du) * np.log1p(np.abs(du))
    vv = np.sign(dv) * np.log1p(np.abs(dv))

    # CPB weights with |w2| folded into layer 2
    w0 = np.asarray(cpb_w0[l], f32)                          # (64,2)
    b0 = np.asarray(cpb_b0[l], f32)                          # (64,)
    W1 = np.asarray(cpb_w1[l], f32)                          # (64,64)
    b1c = np.asarray(cpb_b1[l], f32)
    w2 = np.asarray(cpb_w2[l], f32)[0]                       # (64,)
    aw2 = np.abs(w2)
    W1p = W1 * aw2[:, None]
    b1p = (b1c * aw2)
    sgn = np.sign(w2).astype(f32)
    c2 = float(np.asarray(cpb_b2[l], f32)[0])                # added on host

    cores = []
    for g in range(GROUPS):
        cores.append({
            'xg': x[g].astype(_BF16),
            'wqsT': np.ascontiguousarray((wq5[g] * SCALE).T).astype(_BF16),
            'qs': (q8[g] * SCALE).astype(np.float32),  # numpy-fallback only
            'k': k8[g].astype(_BF16),
            'vT': np.ascontiguousarray(v8[g].T),
            'uf': uu[g].reshape(1, J * W).astype(f32),
            'vf': vv[g].reshape(1, J * H).astype(f32),
            'w0x': np.ascontiguousarray(w0[:, 0].reshape(1, OFF_DIMS)),
            'w0y': np.ascontiguousarray(w0[:, 1].reshape(1, OFF_DIMS)),
            'b0r': b0.reshape(1, OFF_DIMS).copy(),
            'w1pT': np.ascontiguousarray(W1p.T),             # (64,64)
            'b1p': b1p.reshape(OFF_DIMS, 1).copy(),
            'sgn': sgn.reshape(OFF_DIMS, 1).copy(),
        })
    w_out5 = np.asarray(w_out[l], f32)                       # (256,512)
    b_out5 = np.asarray(b_out[l], f32)                       # (256,)
    return cores, w_out5, b_out5, c2


# ---------------------------------------------------------------------------
# Bass program (per core)
# ---------------------------------------------------------------------------

def _build_bass():
    import concourse.bass as bass
    import concourse.mybir as mybir
    from concourse.tile import TileContext

    f32 = mybir.dt.float32
    bf16 = mybir.dt.bfloat16
    AF = mybir.ActivationFunctionType
    ALU = mybir.AluOpType
    AX = mybir.AxisListType

    nc = bass.Bass()
    d_xg = nc.dram_tensor('xg', [32, I], bf16, kind='ExternalInput')
    d_wqsT = nc.dram_tensor('wqsT', [32, 64], bf16, kind='ExternalInput')
    d_k = nc.dram_tensor('k', [64, J], bf16, kind='ExternalInput')
    d_vT = nc.dram_tensor('vT', [J, 64], f32, kind='ExternalInput')
    d_uf = nc.dram_tensor('uf', [1, J * W], f32, kind='ExternalInput')
    d_vf = nc.dram_tensor('vf', [1, J * H], f32, kind='ExternalInput')
    d_w0x = nc.dram_tensor('w0x', [1, 64], f32, kind='ExternalInput')
    d_w0y = nc.dram_tensor('w0y', [1, 64], f32, kind='ExternalInput')
    d_b0r = nc.dram_tensor('b0r', [1, 64], f32, kind='ExternalInput')
    d_w1pT = nc.dram_tensor('w1pT', [64, 64], f32, kind='ExternalInput')
    d_b1p = nc.dram_tensor('b1p', [64, 1], f32, kind='ExternalInput')
    d_sgn = nc.dram_tensor('sgn', [64, 1], f32, kind='ExternalInput')
    d_ident = nc.dram_tensor('ident', [128, 128], f32, kind='ExternalInput')
    d_eye40f = nc.dram_tensor('eye40f', [1, I], f32, kind='ExternalInput')
    f16 = mybir.dt.float16
    d_outT = nc.dram_tensor('outT', [64, I], f16, kind='ExternalOutput')

    with TileContext(nc) as tc:
        with tc.tile_pool(name='const', bufs=1) as cpool, \
             tc.tile_pool(name='work', bufs=4) as wpool, \
             tc.tile_pool(name='p1', bufs=2, space='PSUM') as p1, \
             tc.tile_pool(name='p2', bufs=2, space='PSUM') as p2, \
             tc.tile_pool(name='ps', bufs=2, space='PSUM') as ps, \
             tc.tile_pool(name='pt', bufs=2, space='PSUM') as pt:

            # ---- constant loads -------------------------------------------
            xg_t = cpool.tile([32, I], bf16, tag='xg')
            nc.sync.dma_start(out=xg_t[:], in_=d_xg[:])
            wqsT_t = cpool.tile([32, 64], bf16, tag='wqsT')
            nc.sync.dma_start(out=wqsT_t[:], in_=d_wqsT[:])
            k_t = cpool.tile([64, J], bf16, tag='k')
            nc.sync.dma_start(out=k_t[:], in_=d_k[:])
            vT_t = cpool.tile([J, 64], f32, tag='vT')
            nc.sync.dma_start(out=vT_t[:], in_=d_vT[:])
            uf_t = cpool.tile([1, J * W], f32, tag='uf')
            nc.sync.dma_start(out=uf_t[:], in_=d_uf[:])
            vf_t = cpool.tile([1, J * H], f32, tag='vf')
            nc.sync.dma_start(out=vf_t[:], in_=d_vf[:])
            w0x_t = cpool.tile([1, 64], f32, tag='w0x')
            nc.sync.dma_start(out=w0x_t[:], in_=d_w0x[:])
            w0y_t = cpool.tile([1, 64], f32, tag='w0y')
            nc.sync.dma_start(out=w0y_t[:], in_=d_w0y[:])
            b0r_t = cpool.tile([1, 64], f32, tag='b0r')
            nc.sync.dma_start(out=b0r_t[:], in_=d_b0r[:])
            w1pT_t = cpool.tile([64, 64], f32, tag='w1pT')
            nc.sync.dma_start(out=w1pT_t[:], in_=d_w1pT[:])
            b1p_t = cpool.tile([64, 1], f32, tag='b1p')
            nc.sync.dma_start(out=b1p_t[:], in_=d_b1p[:])
            sgn_t = cpool.tile([64, 1], f32, tag='sgn')
            nc.sync.dma_start(out=sgn_t[:], in_=d_sgn[:])
            id_t = cpool.tile([128, 128], f32, tag='ident')
            nc.sync.dma_start(out=id_t[:], in_=d_ident[:])
            eye40f_t = cpool.tile([1, I], f32, tag='eye40f')
            nc.sync.dma_start(out=eye40f_t[:], in_=d_eye40f[:])

            # ---- device-built constants -----------------------------------
            # S_tile (40,1600): 40 copies of ident40 along free dim
            st_t = cpool.tile([W, I], f32, tag='stile')
            for iy in range(H):
                nc.vector.tensor_copy(st_t[:, iy * W:(iy + 1) * W],
                                      id_t[:W, :W])
            # qs = (wq_g*SCALE)^T-free @ xg, computed on device in bf16
            qs_t = cpool.tile([64, I], bf16, tag='qs')
            for w4 in range(NWIN):
                qp4 = p1.tile([64, WIN], f32, tag='z1')
                nc.tensor.matmul(qp4[:], wqsT_t[:],
                                 xg_t[:, w4 * WIN:(w4 + 1) * WIN],
                                 start=True, stop=True)
                nc.scalar.copy(qs_t[:, w4 * WIN:(w4 + 1) * WIN], qp4[:])

            # ones row (1,40) for the V-build and S_rep matmuls
            ones_t = cpool.tile([1, W], f32, tag='ones')
            nc.vector.memset(ones_t[:], 1.0)
            # S_rep (40,1600): block iy' = e_{iy'} (x) ones(1,40), via PE
            sr_t = cpool.tile([H, I], f32, tag='srep')
            for w4 in range(NWIN):
                srp = ps.tile([H, WIN], f32, tag='simT')
                for dy in range(10):
                    iy = w4 * 10 + dy
                    nc.tensor.matmul(srp[:, dy * W:(dy + 1) * W],
                                     eye40f_t[:, iy * W:(iy + 1) * W],
                                     ones_t[:], start=True, stop=True)
                nc.scalar.copy(sr_t[:, w4 * WIN:(w4 + 1) * WIN], srp[:])
            # SLID (64,199): zeros with column 99 = sgn
            slid_t = cpool.tile([64, 2 * J - 1], f32, tag='slid')
            nc.vector.memset(slid_t[:], 0.0)
            nc.vector.tensor_copy(slid_t[:, J - 1:J], sgn_t[:])

            # ---- U_all / V_all: per-j layer-1 weights ---------------------
            # U_j[ix,c] = u[j,ix] * w0x[c];  V_j[iy,c] = v[j,iy]*w0y[c] + b0[c]
            u_all = cpool.tile([W, J * 64], f32, tag='uall')
            v_all = cpool.tile([H, J * 64], f32, tag='vall')
            for j0 in range(0, J, 4):
                up = p1.tile([W, 4 * 64], f32, tag='z1')
                vp = p2.tile([H, 4 * 64], f32, tag='z2')
                for dj in range(4):
                    j = j0 + dj
                    nc.tensor.matmul(up[:, dj * 64:(dj + 1) * 64],
                                     uf_t[:, j * W:(j + 1) * W],
                                     w0x_t[:], start=True, stop=True)
                    nc.tensor.matmul(vp[:, dj * 64:(dj + 1) * 64],
                                     vf_t[:, j * H:(j + 1) * H],
                                     w0y_t[:], start=True, stop=False)
                    nc.tensor.matmul(vp[:, dj * 64:(dj + 1) * 64],
                                     ones_t[:],
                                     b0r_t[:], start=False, stop=True)
                nc.scalar.copy(u_all[:, j0 * 64:(j0 + 4) * 64], up[:])
                nc.scalar.copy(v_all[:, j0 * 64:(j0 + 4) * 64], vp[:])

            outT_s = cpool.tile([64, I], f16, tag='outT')

            # ---- main loop: windows x j ----------------------------------
            for w in range(NWIN):
                i0 = w * WIN
                simT = ps.tile([J, WIN], f32, tag='simT')
                # attention logits q@k^T (transposed): (j, i)
                nc.tensor.matmul(simT[:], k_t[:], qs_t[:, i0:i0 + WIN],
                                 start=True, stop=False)
                for j in range(J):
                    z1 = p1.tile([64, WIN], f32, tag='z1')
                    nc.tensor.matmul(z1[:], u_all[:, j * 64:(j + 1) * 64],
                                     st_t[:, i0:i0 + WIN], start=True, stop=False)
                    nc.tensor.matmul(z1[:], v_all[:, j * 64:(j + 1) * 64],
                                     sr_t[:, i0:i0 + WIN], start=False, stop=True)
                    h1 = wpool.tile([64, WIN], f32, tag='h1')
                    if j % 2 == 0:
                        nc.scalar.activation(h1[:], z1[:], AF.Relu)
                    else:
                        nc.vector.tensor_scalar_max(h1[:], z1[:], 0.0)
                    z2 = p2.tile([64, WIN], f32, tag='z2')
                    nc.tensor.matmul(z2[:], w1pT_t[:], h1[:],
                                     start=True, stop=True)
                    h2 = wpool.tile([64, WIN], f32, tag='h2')
                    if j % 2 == 0:
                        nc.vector.tensor_scalar(h2[:], z2[:], b1p_t[:], 0.0,
                                                op0=ALU.add, op1=ALU.max)
                    else:
                        nc.scalar.activation(h2[:], z2[:], AF.Relu,
                                             bias=b1p_t[:], scale=1.0)
                    # layer 3: signs land in logits row j
                    nc.tensor.matmul(simT[:], slid_t[:, J - 1 - j:2 * J - 1 - j],
                                     h2[:], start=False, stop=(j == J - 1))

                # ---- softmax over j + attn @ V ---------------------------
                simTs = wpool.tile([J, WIN], f32, tag='simTs')
                nc.vector.tensor_copy(simTs[:], simT[:])
                for s0 in range(0, WIN, 100):
                    trp = pt.tile([128, J], f32, tag='tt')
                    nc.tensor.transpose(trp[:100, :], simTs[:, s0:s0 + 100],
                                        id_t[:J, :J])
                    e_s = wpool.tile([128, J], f32, tag='es')
                    nc.scalar.activation(e_s[:100, :], trp[:100, :], AF.Exp)
                    ssum = wpool.tile([128, 1], f32, tag='ssum')
                    nc.vector.reduce_sum(ssum[:100, :], e_s[:100, :], axis=AX.X)
                    rec = wpool.tile([128, 1], f32, tag='rec')
                    nc.vector.reciprocal(rec[:100, :], ssum[:100, :])
                    nc.vector.tensor_scalar_mul(e_s[:100, :], e_s[:100, :],
                                                rec[:100, :])
                    tr2 = pt.tile([J, 128], f32, tag='tt')
                    nc.tensor.transpose(tr2[:, :100], e_s[:100, :J],
                                        id_t[:100, :100])
                    attTs = wpool.tile([J, 128], f32, tag='attTs')
                    nc.vector.tensor_copy(attTs[:, :100], tr2[:, :100])
                    outTp = pt.tile([64, 128], f32, tag='tt')
                    nc.tensor.matmul(outTp[:, :100], vT_t[:], attTs[:, :100],
                                     start=True, stop=True)
                    nc.scalar.copy(outT_s[:, i0 + s0:i0 + s0 + 100],
                                   outTp[:, :100])

            nc.sync.dma_start(out=d_outT[:], in_=outT_s[:])
    return nc


# ---------------------------------------------------------------------------
# Cached PJRT runner (one jit trace per process; resident zero buffers)
# ---------------------------------------------------------------------------

_RUN = {}
_RUN_LOCK = __import__('threading').Lock()


def _get_runner():
    with _RUN_LOCK:
        return _get_runner_locked()


def _get_runner_locked():
    if 'fn' in _RUN:
        return _RUN['fn']
    import jax
    import concourse.mybir as mybir
    from jax.sharding import Mesh, PartitionSpec, NamedSharding
    from jax.experimental.shard_map import shard_map
    from concourse.bass2jax import _bass_exec_p, install_neuronx_cc_hook
    from concourse.bass2jax import partition_id_tensor
    _install_birfix()
    install_neuronx_cc_hook()

    nc = _build_bass()
    pname = nc.partition_id_tensor.name if nc.partition_id_tensor else None
    in_names, out_names, out_avals, zero_outs = [], [], [], []
    for alloc in nc.m.functions[0].allocations:
        if not isinstance(alloc, mybir.MemoryLocationSet):
            continue
        name = alloc.memorylocations[0].name
        if alloc.kind == 'ExternalInput':
            if name != pname:
                in_names.append(name)
        elif alloc.kind == 'ExternalOutput':
            shape = tuple(alloc.tensor_shape)
            dtype = mybir.dt.np(alloc.dtype)
            out_names.append(name)
            out_avals.append(jax.core.ShapedArray(shape, dtype))
            zero_outs.append(np.zeros(shape, dtype))
    n_params = len(in_names)
    all_names = in_names + out_names + ([pname] if pname else [])

    def _body(*args):
        ops = list(args)
        if pname is not None:
            ops.append(partition_id_tensor())
        return tuple(_bass_exec_p.bind(
            *ops, out_avals=tuple(out_avals), in_names=tuple(all_names),
            out_names=tuple(out_names), lowering_input_output_aliases=(),
            sim_require_finite=True, sim_require_nnan=True, nc=nc))

    devices = jax.devices()[:N_CORES]
    mesh = Mesh(np.asarray(devices), ('core',))
    nargs = n_params + len(out_names)
    sharded = jax.jit(
        shard_map(_body, mesh=mesh, in_specs=(PartitionSpec('core'),) * nargs,
                  out_specs=(PartitionSpec('core'),) * len(out_names),
                  check_rep=False),
        keep_unused=True)
    sh = NamedSharding(mesh, PartitionSpec('core'))
    zres = [jax.device_put(
        np.zeros((N_CORES * z.shape[0], *z.shape[1:]), z.dtype), sh)
        for z in zero_outs]
    # call-invariant constants: uploaded once, kept resident on device
    _statics = {
        'ident': np.eye(128, dtype=np.float32),
        'eye40f': np.eye(W, dtype=np.float32).reshape(1, I),
    }
    static_res = {
        name: jax.device_put(np.concatenate([arr] * N_CORES, axis=0), sh)
        for name, arr in _statics.items()
    }

    def run(cores):
        # async uploads issued ahead of dispatch: the execute RPC then
        # carries no inline host buffers
        concat = [static_res[name] if name in static_res else
                  jax.device_put(np.concatenate(
                      [np.asarray(c[name]) for c in cores], axis=0), sh)
                  for name in in_names]
        outs = sharded(*concat, *zres)
        res = []
        for i, name in enumerate(out_names):
            arr = np.asarray(outs[i]).reshape(N_CORES, *out_avals[i].shape)
            res.append((name, arr))
        return dict(res)

    _RUN['fn'] = run
    return run


# ---------------------------------------------------------------------------
# numpy fallback (BLAS-based; used only if the device path fails)
# ---------------------------------------------------------------------------

def _cpb_attn_numpy(cores):
    f32 = np.float32
    G = len(cores)
    uu = np.stack([c['uf'].reshape(J, W) for c in cores])      # (8,J,W)
    vv = np.stack([c['vf'].reshape(J, H) for c in cores])
    xb = np.empty((G, J, H, W, 2), f32)
    xb[..., 0] = uu[:, :, None, :]
    xb[..., 1] = vv[:, :, :, None]
    xb = xb.reshape(-1, 2)                                     # (8*J*I, 2)
    w0 = np.concatenate([cores[0]['w0x'], cores[0]['w0y']], axis=0)  # (2,64)
    z1 = xb @ w0
    z1 += cores[0]['b0r'][0][None, :]
    np.maximum(z1, 0.0, out=z1)
    z2 = z1 @ cores[0]['w1pT']
    z2 += cores[0]['b1p'][:, 0][None, :]
    np.maximum(z2, 0.0, out=z2)
    bias = (z2 @ cores[0]['sgn'][:, 0]).reshape(G, J, I)
    outs = []
    for g, cin in enumerate(cores):
        sim = (cin['k'].astype(f32).T @ cin['qs'].astype(f32) + bias[g])
        e = np.exp(sim - sim.max(axis=0, keepdims=True))
        att = e / e.sum(axis=0, keepdims=True)
        outs.append(cin['vT'].T @ att)                         # (64,I)
    return outs


def kernel(**inputs):
    inputs = {k: np.asarray(v) for k, v in inputs.items()}
    cores, w_out5, b_out5, c2 = _host_prep(**inputs)
    try:
        run = _get_runner()
        res = run(cores)
        outT = res['outT'].astype(np.float32)                 # (8,64,I)
    except Exception:
        import traceback; traceback.print_exc()
        outT = np.stack(_cpb_attn_numpy(cores))
    full = outT.reshape(INNER, I)
    out = _BUF.setdefault('proj', np.empty((D_MODEL, I), np.float32))
    np.matmul(w_out5, full, out=out)
    out += b_out5[:, None]
    return out.reshape(1, D_MODEL, H, W).astype(np.float32, copy=True)


def _warmup():
    try:
        run = _get_runner()
        # one full dummy invocation so the first real call hits steady state
        z = {'xg': np.zeros((32, I), _BF16),
             'wqsT': np.zeros((32, 64), _BF16),
             'k': np.zeros((64, J), _BF16),
             'vT': np.zeros((J, 64), np.float32),
             'uf': np.zeros((1, J * W), np.float32),
             'vf': np.zeros((1, J * H), np.float32),
             'w0x': np.zeros((1, 64), np.float32),
             'w0y': np.zeros((1, 64), np.float32),
             'b0r': np.zeros((1, 64), np.float32),
             'w1pT': np.zeros((64, 64), np.float32),
             'b1p': np.zeros((64, 1), np.float32),
             'sgn': np.zeros((64, 1), np.float32)}
        run([z] * N_CORES)
    except Exception:
        pass


_WARM_THREAD = __import__('threading').Thread(target=_warmup, daemon=True)
_WARM_THREAD.start()
